# revision 13
# baseline (speedup 1.0000x reference)
"""AdaptedAttention (llama-adapter) Trainium2 kernel, 8-core token-data-parallel.

v3 strategy:
  - Adapter K/V and the V@o_w fold are computed on HOST (they depend only on
    weights/prompt, not activations).  This removes the device adapter phase
    and the AllGather entirely, and halves the o-proj contraction:
        adapter_out = probs @ W2,  W2[h*L+l, :] = adapter_v[h,l,:] @ o_w_scaled[h]
    so the second GEMM contracts 2048 instead of 4096.
  - Per-core device work: q-proj GEMM (bf16), RoPE, per-head-pair scores
    against host-computed adapter K, exp/denominator/normalize, probs @ W2
    GEMM (bf16).  base_output + o_bias are added on host after the run.
  - Heads processed in pairs so softmax vector work runs on [128, TOK] tiles.
  - Scores matmuls are software-pipelined two heads behind the q-proj stream
    so the PE never waits on the DVE RoPE chain.
  - Weights land with ONE contiguous DMA per m-tile (4 KB/partition bursts);
    first two heads' q-proj is k-interleaved to overlap the initial x load.
  - reciprocal_approx_fast for the softmax denominator (DVE InstReciprocal is
    ~6.5us/tile; the approx is ~1.3us and 18 bits is plenty).
"""
import os
import numpy as np

import concourse.bass as bass
import concourse.tile as tile
from concourse import mybir
from concourse.bass_utils import run_bass_kernel_spmd

F32 = mybir.dt.float32
BF16 = mybir.dt.bfloat16
P = 128

NUM_HEADS = 32
B, S, H, L = 4, 2048, 4096, 64
HD = H // NUM_HEADS            # 128
NC = 8
TOK = (B * S) // NC            # 1024 tokens per core
KT = H // P                    # 32 contraction tiles (q-proj)
MT = H // P                    # 32 output feature tiles
K2 = (NUM_HEADS * L) // P      # 16 contraction tiles (W2 GEMM)
NB = TOK // 512                # 2 moving-operand chunks of 512

_PATCHED = False


def _patch_tile():
    """TRN2 instructions have one hw wait slot; walrus rejects multi-wait
    matmuls and the kernel-tail drain. Hoist extra waits onto NoOps."""
    global _PATCHED
    if _PATCHED:
        return
    _PATCHED = True
    import concourse.tile as _tile
    from concourse.vector_clock import ScopedClock

    _orig_commit = _tile.TileContext._commit_instruction

    def _patched_commit(self, inst, lazy_reg_writes=True):
        si = getattr(inst, "sync_info", None)
        if (
            si is not None
            and si.on_wait
            and len(si.on_wait) > 1
            and inst.engine != mybir.EngineType.Unassigned
            and not isinstance(inst, mybir.InstNoOp)
        ):
            waits = list(si.on_wait)
            for w in waits[:-1]:
                nop = mybir.InstNoOp(
                    name=self.nc.get_next_instruction_name(),
                    ins=[], outs=[], bass_nofuse=True,
                )
                nop.engine = inst.engine
                nop.sync_info = mybir.SyncInfo(on_wait=[w], on_update=[])
                _orig_commit(self, nop, lazy_reg_writes=False)
            inst.sync_info = mybir.SyncInfo(
                on_wait=waits[-1:], on_update=list(si.on_update or [])
            )
        return _orig_commit(self, inst, lazy_reg_writes=lazy_reg_writes)

    def _patched_drain_and_barrier(self, tick_clock, wait_clock):
        nc = self.nc
        carrier = nc.sync.nop(nofuse=True)
        wait_clock.add_sem_waits(
            carrier.ins, ScopedClock({None: tick_clock.global_clock})
        )
        si = carrier.ins.sync_info
        waits = list(si.on_wait) if si and si.on_wait else []
        if len(waits) > 1:
            carrier.ins.sync_info = mybir.SyncInfo(
                on_wait=waits[:1], on_update=list(si.on_update or [])
            )
            for w in waits[1:]:
                extra = nc.sync.nop(nofuse=True)
                extra.ins.sync_info = mybir.SyncInfo(on_wait=[w], on_update=[])
        nc.sync.drain()
        nc.all_engine_barrier()
        assert self.sems is not None
        popped = nc._tile_sem_poison_stack.pop()
        assert popped is self._sem_poison
        nc.clear_and_free_semaphores(list(self.sems.allocated().values()))
        nc.all_engine_barrier()

    _tile.TileContext._commit_instruction = _patched_commit
    _tile.TileContext._drain_and_barrier = _patched_drain_and_barrier


def build_nc():
    _patch_tile()
    from contextlib import ExitStack

    nc = bass.Bass(target_bir_lowering=False)

    xT = nc.declare_dram_parameter("xT", [H, TOK], BF16, isOutput=False)
    qwB = nc.declare_dram_parameter("qwB", [MT, P, KT * P], BF16,
                                    isOutput=False)
    w2B = nc.declare_dram_parameter("w2B", [MT, P, K2 * P], BF16,
                                    isOutput=False)
    kattD = nc.declare_dram_parameter("kattD", [HD, NUM_HEADS * L], BF16,
                                      isOutput=False)
    cosT = nc.declare_dram_parameter("cosT", [HD, TOK], BF16, isOutput=False)
    srotT = nc.declare_dram_parameter("srotT", [HD, TOK], BF16, isOutput=False)
    qb2 = nc.declare_dram_parameter("qb2", [P, MT], F32, isOutput=False)
    qbr = nc.declare_dram_parameter("qbr", [P, MT], F32, isOutput=False)
    outT = nc.declare_dram_parameter("outT", [H, TOK], BF16, isOutput=True)

    with tile.TileContext(nc) as tc:
        es = ExitStack()
        persist = es.enter_context(tc.tile_pool(name="persist", bufs=1))
        psA = es.enter_context(tc.tile_pool(name="psA", bufs=2, space="PSUM"))
        psB = es.enter_context(tc.tile_pool(name="psB", bufs=2, space="PSUM"))
        wpool = es.enter_context(tc.tile_pool(name="wpool", bufs=3))
        rpool = es.enter_context(tc.tile_pool(name="rpool", bufs=2))

        # ---- DMA ordering: weights for heads 0/1 + small tables + x ----
        qw_tiles = {}

        def issue_qw(m):
            wb = wpool.tile([P, KT, P], BF16, name=f"qw_{m}", tag="qw")
            (nc.sync, nc.gpsimd)[m % 2].dma_start(out=wb[:], in_=qwB[m])
            qw_tiles[m] = wb

        issue_qw(0)
        cos_sb = persist.tile([HD, TOK], BF16, name="cos_sb")
        srot_sb = persist.tile([HD, TOK], BF16, name="srot_sb")
        katt = persist.tile([HD, NUM_HEADS * L], BF16, name="katt")
        qb2_sb = persist.tile([P, MT], F32, name="qb2_sb")
        qbr_sb = persist.tile([P, MT], F32, name="qbr_sb")
        nc.scalar.dma_start(out=qb2_sb[:], in_=qb2[:])
        nc.scalar.dma_start(out=qbr_sb[:], in_=qbr[:])
        nc.scalar.dma_start(out=cos_sb[:], in_=cosT[:])
        nc.scalar.dma_start(out=srot_sb[:], in_=srotT[:])
        issue_qw(1)
        xt_tiles = []
        for k in range(KT):
            t = persist.tile([P, TOK], BF16, name=f"xt_{k}")
            (nc.sync, nc.scalar, nc.gpsimd)[k % 3].dma_start(
                out=t[:], in_=xT[k * P:(k + 1) * P, :])
            xt_tiles.append(t)
        nc.scalar.dma_start(out=katt[:], in_=kattD[:])

        # block-diagonal ones (denominator matmul for a head pair)
        onesbd = persist.tile([P, P], BF16, name="onesbd")
        nc.vector.memset(onesbd[:], 0.0)
        nc.vector.memset(onesbd[0:64, 0:64], 1.0)
        nc.vector.memset(onesbd[64:128, 64:128], 1.0)

        # normalized probs for the W2 GEMM: [l-in-pair(128), pair(16), tok]
        probs = persist.tile([P, K2, TOK], BF16, name="probs")

        pq_tiles = {}
        qr_tiles = {}
        expt_tiles = {}
        sc_tiles = {}

        def emit_mm(h, k):
            wb = qw_tiles[h]
            for j in range(NB):
                nc.tensor.matmul(
                    out=pq_tiles[h][:, j * 512:(j + 1) * 512],
                    lhsT=wb[:, k, :],
                    rhs=xt_tiles[k][:, j * 512:(j + 1) * 512],
                    start=(k == 0), stop=(k == KT - 1),
                )

        def emit_rope(h):
            pq = pq_tiles.pop(h)
            q_sb = rpool.tile([P, TOK], F32, name=f"q_sb_{h}", tag="q_sb")
            nc.scalar.copy(out=q_sb[:], in_=pq[:])
            rot = rpool.tile([P, TOK], F32, name=f"rot_{h}", tag="rot")
            nc.sync.dma_start(out=rot[0:64, :], in_=q_sb[64:128, :])
            nc.gpsimd.dma_start(out=rot[64:128, :], in_=q_sb[0:64, :])
            t2 = rpool.tile([P, TOK], F32, name=f"t2_{h}", tag="t2")
            nc.vector.scalar_tensor_tensor(
                out=t2[:], in0=q_sb[:], scalar=qb2_sb[:, h:h + 1],
                in1=cos_sb[:], op0=mybir.AluOpType.add,
                op1=mybir.AluOpType.mult)
            t1 = rpool.tile([P, TOK], F32, name=f"t1_{h}", tag="t1")
            nc.vector.scalar_tensor_tensor(
                out=t1[:], in0=rot[:], scalar=qbr_sb[:, h:h + 1],
                in1=srot_sb[:], op0=mybir.AluOpType.add,
                op1=mybir.AluOpType.mult)
            qr = rpool.tile([P, TOK], BF16, name=f"qr_{h}", tag="qr", bufs=3)
            nc.vector.tensor_add(out=qr[:], in0=t2[:], in1=t1[:])
            qr_tiles[h] = qr

        def emit_scores(h):
            p_, par = divmod(h, 2)
            if par == 0:
                sc_tiles[p_] = psB.tile([P, TOK], F32, name=f"sc_{p_}",
                                        tag="psB")
            sc = sc_tiles[p_]
            qr = qr_tiles.pop(h)
            for j in range(NB):
                nc.tensor.matmul(
                    out=sc[par * 64:(par + 1) * 64, j * 512:(j + 1) * 512],
                    lhsT=katt[:, h * L:(h + 1) * L],
                    rhs=qr[:, j * 512:(j + 1) * 512],
                    start=True, stop=True,
                )
            if par == 1:
                expt = rpool.tile([P, TOK], BF16, name=f"expt_{p_}",
                                  tag="expt")
                nc.scalar.activation(expt[:], sc[:],
                                     mybir.ActivationFunctionType.Exp)
                expt_tiles[p_] = expt

        def emit_den(p_):
            den = psB.tile([P, TOK], F32, name=f"den_{p_}", tag="psB")
            expt = expt_tiles[p_]
            for j in range(NB):
                nc.tensor.matmul(
                    out=den[:, j * 512:(j + 1) * 512],
                    lhsT=onesbd[:],
                    rhs=expt[:, j * 512:(j + 1) * 512],
                    start=True, stop=True,
                )
            recip = rpool.tile([P, TOK], F32, name=f"recip_{p_}", tag="recip")
            nc.vector.reciprocal(out=recip[:], in_=den[:])
            nc.vector.tensor_mul(out=probs[:, p_, :], in0=expt_tiles.pop(p_),
                                 in1=recip[:])

        # ---- heads 0-2 k-interleaved (overlaps the initial x stream); the
        # third PSUM accumulator borrows psB, which is idle until scores ----
        pq_tiles[0] = psA.tile([P, TOK], F32, name="pq_0", tag="psA")
        pq_tiles[1] = psA.tile([P, TOK], F32, name="pq_1", tag="psA")
        pq_tiles[2] = psB.tile([P, TOK], F32, name="pq_2", tag="psB")
        issue_qw(2)
        issue_qw(3)
        for k in range(KT):
            emit_mm(0, k)
            emit_mm(1, k)
            emit_mm(2, k)
        emit_rope(0)
        emit_rope(1)
        emit_rope(2)

        # ---- phase-2 weight prefetch plumbing (issued in the phase-1 tail) ----
        w2_tiles = {}

        def issue_w2(m):
            wb = wpool.tile([P, K2, P], BF16, name=f"w2_{m}", tag="w2")
            (nc.sync, nc.gpsimd)[m % 2].dma_start(out=wb[:], in_=w2B[m])
            w2_tiles[m] = wb

        # ---- heads 3..31, scores lagging three heads behind ----
        for it in range(3, MT + 3):
            if it == MT - 1:
                issue_w2(0)
                issue_w2(1)
            if it < MT:
                pq_tiles[it] = psA.tile([P, TOK], F32, name=f"pq_{it}",
                                        tag="psA")
                if it + 1 < MT:
                    issue_qw(it + 1)
                for k in range(KT // 2):
                    emit_mm(it, k)
            emit_scores(it - 3)
            if it >= 5 and it % 2 == 1:
                emit_den((it - 5) // 2)
            if it < MT:
                for k in range(KT // 2, KT):
                    emit_mm(it, k)
                qw_tiles.pop(it, None)
                emit_rope(it)

        # ---- phase 2: probs @ W2 (den of the last pair overlaps m=0/1,
        # whose k-loops are interleaved to cover the last recip/normalize) ----
        emit_den(K2 - 1)

        po_tiles = {}

        def emit_w2_mm(m, k):
            wb = w2_tiles[m]
            for j in range(NB):
                nc.tensor.matmul(
                    out=po_tiles[m][:, j * 512:(j + 1) * 512],
                    lhsT=wb[:, k, :],
                    rhs=probs[:, k, j * 512:(j + 1) * 512],
                    start=(k == 0), stop=(k == K2 - 1),
                )

        def emit_out(m, split=False):
            po = po_tiles.pop(m)
            w2_tiles.pop(m)
            os_ = rpool.tile([P, TOK], BF16, name=f"os_{m}", tag="os", bufs=3)
            if split:
                nc.scalar.copy(out=os_[:, 0:512], in_=po[:, 0:512])
                nc.scalar.copy(out=os_[:, 512:1024], in_=po[:, 512:1024])
                (nc.sync, nc.scalar, nc.gpsimd)[m % 3].dma_start(
                    out=outT[m * P:(m + 1) * P, 0:512], in_=os_[:, 0:512])
                (nc.sync, nc.scalar, nc.gpsimd)[(m + 1) % 3].dma_start(
                    out=outT[m * P:(m + 1) * P, 512:1024],
                    in_=os_[:, 512:1024])
            else:
                nc.scalar.copy(out=os_[:], in_=po[:])
                (nc.sync, nc.scalar, nc.gpsimd)[m % 3].dma_start(
                    out=outT[m * P:(m + 1) * P, :], in_=os_[:])

        po_tiles[0] = psA.tile([P, TOK], F32, name="po_0", tag="psA")
        po_tiles[1] = psA.tile([P, TOK], F32, name="po_1", tag="psA")
        issue_w2(2)
        for k in range(K2):
            emit_w2_mm(0, k)
            emit_w2_mm(1, k)
        emit_out(0)
        emit_out(1)
        for m in range(2, MT):
            if m + 1 < MT:
                issue_w2(m + 1)
            pool_ = psB if m == 2 else psA
            po_tiles[m] = pool_.tile([P, TOK], F32, name=f"po_{m}",
                                     tag="psB" if m == 2 else "psA")
            for k in range(K2):
                emit_w2_mm(m, k)
            emit_out(m, split=(m == MT - 1))

        es.close()
    return nc


_NC_CACHE = None


def kernel(hidden_states, position_ids, base_output, cos, sin,
           q_w, k_w, v_w, o_w,
           q_scale, k_scale, v_scale, o_scale,
           q_bias, k_bias, v_bias, o_bias,
           adaption_prompt, adaption_gate):
    global _NC_CACHE
    import ml_dtypes

    hidden_states = np.asarray(hidden_states, dtype=np.float32)
    base_output = np.asarray(base_output, dtype=np.float32)
    pos = np.asarray(position_ids).reshape(-1).astype(np.int64)
    cos = np.asarray(cos, dtype=np.float32)
    sin = np.asarray(sin, dtype=np.float32)

    X = hidden_states.reshape(B * S, H)
    BASE = base_output.reshape(B * S, H) + np.asarray(o_bias, np.float32)[None, :]
    qb2_ = np.ascontiguousarray(
        np.asarray(q_bias, dtype=np.float32).reshape(MT, P).T)
    qbr_ = np.ascontiguousarray(np.roll(qb2_, -64, axis=0))
    cosg = cos[0, 0][pos]                     # [B*S, HD]
    sing = sin[0, 0][pos]
    sr = sing.copy()
    sr[:, :HD // 2] *= -1.0                   # sign for rotate_half product

    inv = 1.0 / np.sqrt(HD)
    gate = float(np.asarray(adaption_gate).reshape(-1)[0])
    q_wT = np.ascontiguousarray(
        (np.asarray(q_w, np.float32) * np.asarray(q_scale, np.float32)[None, :]).T)

    # host-side adapter K/V and the V @ o_w fold
    pr = np.asarray(adaption_prompt, np.float32)[0]            # [L, H]
    ak = (pr * np.asarray(k_scale, np.float32)) @ np.asarray(k_w, np.float32).T \
        + np.asarray(k_bias, np.float32)
    av = (pr * np.asarray(v_scale, np.float32)) @ np.asarray(v_w, np.float32).T \
        + np.asarray(v_bias, np.float32)
    ak = ak.reshape(L, NUM_HEADS, HD).transpose(1, 0, 2) * inv   # [nh, L, hd]
    av = av.reshape(L, NUM_HEADS, HD).transpose(1, 0, 2) * gate
    o_ws = (np.asarray(o_w, np.float32)
            * np.asarray(o_scale, np.float32)[None, :]).T        # [H(in), H(out)]
    W2 = np.matmul(av, o_ws.reshape(NUM_HEADS, HD, H))   # [nh, L, H]
    W2f = np.ascontiguousarray(W2.reshape(NUM_HEADS * L, H))     # [2048, H]
    katt_ = np.ascontiguousarray(
        ak.transpose(2, 0, 1).reshape(HD, NUM_HEADS * L))        # [128, 2048]

    # blocked stationary layouts: [m, 128(part), kg, kk, 128]
    qwB_ = np.ascontiguousarray(
        q_wT.reshape(8, 4, P, MT, P).transpose(3, 2, 0, 1, 4)).reshape(
            MT, P, KT * P).astype(ml_dtypes.bfloat16)
    w2B_ = np.ascontiguousarray(
        W2f.reshape(4, 4, P, MT, P).transpose(3, 2, 0, 1, 4)).reshape(
            MT, P, K2 * P).astype(ml_dtypes.bfloat16)
    katt_ = katt_.astype(ml_dtypes.bfloat16)

    in_maps = []
    for c in range(NC):
        sl = slice(c * TOK, (c + 1) * TOK)
        in_maps.append({
            "xT": np.ascontiguousarray(X[sl].T).astype(ml_dtypes.bfloat16),
            "cosT": np.ascontiguousarray(cosg[sl].T).astype(ml_dtypes.bfloat16),
            "srotT": np.ascontiguousarray(sr[sl].T).astype(ml_dtypes.bfloat16),
            "qwB": qwB_,
            "w2B": w2B_,
            "kattD": katt_,
            "qb2": qb2_,
            "qbr": qbr_,
        })

    if _NC_CACHE is None:
        _NC_CACHE = build_nc()
    nc = _NC_CACHE

    trace = bool(os.environ.get("KERNEL_TRACE"))
    res = run_bass_kernel_spmd(nc, in_maps, core_ids=list(range(NC)),
                               trace=trace)
    if trace and res.exec_time_ns is not None:
        print(f"HW exec time: {res.exec_time_ns} ns")

    out = np.empty((B * S, H), dtype=np.float32)
    for c in range(NC):
        out[c * TOK:(c + 1) * TOK, :] = res.results[c]["outT"].T.astype(np.float32)
    out += BASE
    return out.reshape(B, S, H)
